# revision 29
# baseline (speedup 1.0000x reference)
"""Trainium2 Bass kernel for nn_AsymmetricLossCustomPriorityRankNewNegOne.

Pure data parallel across 8 NeuronCores: core i takes rows [i*512, (i+1)*512);
each core reduces its rows to a partial scalar on-device and the host adds the
8 partials (the trivial all-reduce).

Only columns [0, 1000) of the 9605-wide inputs are ever used (the whitelist
masks cover exactly those); sigmoid is monotone so all masked maxes run on raw
x and sigmoid is applied to per-row scalars only.

v2 structure (per 128-row block, 4 blocks per core):
  * x ships as bf16 with each group's columns split into two contiguous
    25-col halves (A|B layout, host-side per-tensor relayout), so a 2x-mode
    DVE tensor_tensor max folds 1000 -> 500 before the s=25 group reduce.
    The halving runs on the (otherwise idle) GpSimd engine.
  * y_neg ships as an int8 additive mask m in {0,-64}; one SWDGE cast-DMA
    expands it to bf16 on the way into SBUF. The whole wrong-column pass is
    ONE fused DVE tensor_tensor_reduce: accum = max(x + m) per row.
    (Masked columns sit at x-64 < any unmasked x, and an empty wrong-set
    degenerates to sigmoid(-59) ~ 0, matching the reference's NEG_INF.)
  * y ships as packbits words (two 25-bit int32 words per 50-col group,
    lossless re-encoding); the presence OR is an s=2 max reduce on device.
  * first-present-group selection via priority encoding
    val[l] = present[l] * ((L-l)*32 + gmax[l] + 8); max_l val picks the
    lowest present l and mod(mval,32)-8 recovers its group max.
  * epilogue is three short parallel chains (rank_wl / rank_other halves)
    with compare thresholds hoisted to pre-sigmoid space (logit(0.55) etc.)
    so the ACT sigmoids and DVE compares run concurrently; the partition sum
    runs on the PE with the final accumulation in the ACT copy.
"""

import numpy as np
import sys
from contextlib import ExitStack

sys.path.insert(0, "/opt/trn_rl_repo")

import concourse.bass as bass
import concourse.bacc as bacc
import concourse.mybir as mybir
import concourse.tile as tile
from concourse.bass_utils import run_bass_kernel_spmd

B, C = 4096, 9605
L, G = 20, 50
CU = L * G          # 1000 used columns
H = CU // 2         # 500 (A|B halves)
NCORES = 8
RPC = B // NCORES   # 512 rows per core
PB = RPC // 128     # 4 partition blocks of 128 rows

F32 = mybir.dt.float32
I32 = mybir.dt.int32
I8 = mybir.dt.int8
BF16 = mybir.dt.bfloat16
AX = mybir.AxisListType.X
OP = mybir.AluOpType
ACT = mybir.ActivationFunctionType

LOGIT_55 = 0.2006707  # logit(0.55); logit(0.45) = -LOGIT_55
MNEG = -64.0          # additive mask for non-wrong columns


K_LSE = 22.0   # exp sharpness for the ACT-engine wrong-col log-sum-exp
C_LSE = 47.0   # exp offset: max arg ~ 22*5.6-47 = 76 keeps fp32 sums < 1.3e36
LN2 = 0.6931471805599453
BITHACK_BIAS = -126.94269504  # log2(u) ~ float(bits(u))*2^-23 + BITHACK_BIAS


def build_nc(reps=1, loop_n=None, variant="v3"):
    lse = variant.startswith("v5") or variant.startswith("v6")
    tanh_epi = variant.startswith("v6")
    m_bf16_wire = "w" in variant or lse
    n_pool_adds = 0
    if "p1" in variant:
        n_pool_adds = 1
    elif "p2" in variant:
        n_pool_adds = 2
    nc = bacc.Bacc()
    x_ext = nc.declare_dram_parameter("x", [RPC, CU], BF16, isOutput=False)
    m_ext = nc.declare_dram_parameter(
        "m", [128, PB * CU], BF16 if m_bf16_wire else I8, isOutput=False
    )
    yp_ext = nc.declare_dram_parameter("y_p", [128, PB * L * 2], I32, isOutput=False)
    out_ext = nc.declare_dram_parameter("out", [1, 1], F32, isOutput=True)

    # which blocks route the wrong-pass add through Pool (Pool TT supports
    # add/mult only — max must stay on DVE)
    pool_add = tuple(n >= PB - n_pool_adds for n in range(PB))

    with ExitStack() as ctx:
        tc = ctx.enter_context(tile.TileContext(nc))
        const_pool = ctx.enter_context(tc.tile_pool(name="const", bufs=1))
        in_pool = ctx.enter_context(tc.tile_pool(name="inp", bufs=3))
        mid_pool = ctx.enter_context(tc.tile_pool(name="mid", bufs=3))
        acc_pool = ctx.enter_context(tc.tile_pool(name="acc", bufs=2))
        psum_pool = ctx.enter_context(tc.tile_pool(name="psum", bufs=1, space="PSUM"))

        # constants
        prio80 = const_pool.tile([128, PB * L], F32)
        nc.gpsimd.iota(
            prio80[:], pattern=[[0, PB], [-32, L]], base=int(L * 32 + 8),
            channel_multiplier=0, allow_small_or_imprecise_dtypes=True,
        )
        ones = const_pool.tile([128, 1], F32)
        nc.vector.memset(ones[:], 1.0)
        # dummy activation first: pins the ACT table set used by the kernel
        # (sigmoid set for v3, ln+exp set for v5) so no mid-kernel
        # LoadActFuncSet reload occurs
        actwarm = const_pool.tile([1, 1], F32)
        if tanh_epi:
            nc.scalar.activation(actwarm[:], ones[0:1, 0:1], ACT.Exp)
        elif lse:
            nc.scalar.activation(actwarm[:], ones[0:1, 0:1], ACT.Ln)
            nc.scalar.activation(actwarm[:], ones[0:1, 0:1], ACT.Exp)
        else:
            nc.scalar.activation(actwarm[:], ones[0:1, 0:1], ACT.Sigmoid)
        bm8 = const_pool.tile([128, 1], F32)
        nc.vector.memset(bm8[:], -8.0)
        b55 = const_pool.tile([128, 1], F32)
        nc.vector.memset(b55[:], 5.5)
        bm45 = const_pool.tile([128, 1], F32)
        nc.vector.memset(bm45[:], -4.5)
        b8 = const_pool.tile([128, 1], F32)
        nc.vector.memset(b8[:], 8.0)
        bm55e = const_pool.tile([128, 1], F32)
        nc.vector.memset(bm55e[:], -5.5)
        b45 = const_pool.tile([128, 1], F32)
        nc.vector.memset(b45[:], 4.5)
        bmC = const_pool.tile([128, 1], F32)
        nc.vector.memset(bmC[:], -C_LSE)
        bmCk = const_pool.tile([128, 1], F32)
        nc.vector.memset(bmCk[:], -C_LSE / K_LSE)
        bm4 = const_pool.tile([128, 1], F32)
        nc.vector.memset(bm4[:], -4.0)
        b025 = const_pool.tile([128, 1], F32)
        nc.vector.memset(b025[:], 0.25)
        bC2K = const_pool.tile([128, 1], F32)
        nc.vector.memset(bC2K[:], C_LSE / (2 * K_LSE))
        bz = const_pool.tile([128, 1], F32)
        nc.vector.memset(bz[:], 0.0)

        import contextlib
        loop_cm = tc.For_i(0, loop_n, 1) if loop_n else contextlib.nullcontext()
        with loop_cm:
          for _rep in range(reps):
            # ---- tiles
            xts = [
                in_pool.tile([128, CU], BF16, tag=f"xt{n}", name=f"xt{n}")
                for n in range(PB)
            ]
            mt = acc_pool.tile([128, PB * CU], BF16)
            ypt = acc_pool.tile([128, PB * L * 2], I32)
            gm_all = acc_pool.tile([128, PB * L], F32)
            ygm = acc_pool.tile([128, PB * L], F32)
            vala = acc_pool.tile([128, PB * L], F32)
            t1a = acc_pool.tile([128, PB * L], F32)
            w = acc_pool.tile([128, PB], F32)

            # ---- DMAs: x block 0 first (DVE's first dependency); per-block
            # int8 mask slices interleave with the x blocks so each block's
            # operands land (and its ACT i8->bf16 cast runs) just in time.
            mi = acc_pool.tile([128, PB * CU], I8, name="mi")
            mdst = mt if m_bf16_wire else mi
            nc.sync.dma_start(xts[0][:], x_ext[bass.ts(0, 128), :])
            nc.sync.dma_start(mdst[:, bass.ts(0, CU)], m_ext[:, bass.ts(0, CU)])
            for n in range(1, PB):
                nc.sync.dma_start(xts[n][:], x_ext[bass.ts(n, 128), :])
                nc.sync.dma_start(mdst[:, bass.ts(n, CU)], m_ext[:, bass.ts(n, CU)])
            nc.sync.dma_start(ypt[:], yp_ext[:])

            # mask expansion i8 -> bf16 on the (otherwise idle) ACT engine
            if not m_bf16_wire:
                for n in range(PB):
                    if not pool_add[n]:
                        nc.scalar.activation(
                            mt[:, bass.ts(n, CU)], mi[:, bass.ts(n, CU)], ACT.Copy
                        )

            # ---- presence OR: one s=2 max reduce over the packed words
            nc.vector.tensor_reduce(
                ygm[:], ypt[:].rearrange("p (m s) -> p m s", s=2),
                axis=AX, op=OP.max,
            )

            # ---- per-block: wrong pass (add + halve + reduce) and
            # group-max pass (halve + s=25 reduce), all 2x-mode where possible
            for n in range(PB):
                xt = xts[n]
                if pool_add[n]:
                    # Pool computes S = x + m (mixed dtypes, software convert);
                    # DVE halves + reduces it
                    sp = mid_pool.tile([128, CU], F32, tag=f"sp{n % 2}")
                    nc.gpsimd.tensor_tensor(
                        sp[:], xt[:], mi[:, bass.ts(n, CU)], op=OP.add
                    )
                    sh = mid_pool.tile([128, H], F32, tag=f"shp{n % 2}")
                    nc.vector.tensor_tensor(sh[:], sp[:, :H], sp[:, H:], op=OP.max)
                    nc.vector.tensor_reduce(
                        w[:, n : n + 1], sh[:], axis=AX, op=OP.max
                    )
                elif lse:
                    scr = mid_pool.tile([128, CU], BF16, tag=f"scr{n % 2}")
                    nc.vector.tensor_tensor(
                        scr[:], xt[:], mt[:, bass.ts(n, CU)], op=OP.add
                    )
                    # fused exp + row-sum on ACT: wsum = sum exp(K*S - C);
                    # masked cols underflow to exactly 0
                    escr = mid_pool.tile([128, CU], BF16, tag=f"escr{n % 2}")
                    nc.scalar.activation(
                        escr[:], scr[:], ACT.Exp, scale=K_LSE, bias=bmC[:],
                        accum_out=w[:, n : n + 1],
                    )
                else:
                    scr = mid_pool.tile([128, CU], BF16, tag=f"scr{n % 2}")
                    nc.vector.tensor_tensor(
                        scr[:], xt[:], mt[:, bass.ts(n, CU)], op=OP.add
                    )
                    sh = mid_pool.tile([128, H], BF16, tag=f"sh{n % 2}")
                    nc.vector.tensor_tensor(sh[:], scr[:, :H], scr[:, H:], op=OP.max)
                    nc.vector.tensor_reduce(
                        w[:, n : n + 1], sh[:], axis=AX, op=OP.max
                    )
                xh = mid_pool.tile([128, H], BF16, tag=f"xhv{n % 2}")
                nc.vector.tensor_tensor(xh[:], xt[:, :H], xt[:, H:], op=OP.max)
                nc.vector.tensor_reduce(
                    gm_all[:, bass.ts(n, L)],
                    xh[:].rearrange("p (g s) -> p g s", s=G // 2),
                    axis=AX, op=OP.max,
                )
                # priority add off the critical path (Pool)
                nc.gpsimd.tensor_tensor(
                    t1a[:, bass.ts(n, L)], gm_all[:, bass.ts(n, L)],
                    prio80[:, bass.ts(n, L)], op=OP.add,
                )

            # ---- batched priority-encode + row stats
            mno = acc_pool.tile([128, PB], F32, tag="mno")
            nc.vector.tensor_reduce(
                mno[:], gm_all[:].rearrange("p (n l) -> p n l", l=L),
                axis=AX, op=OP.max,
            )
            nc.vector.scalar_tensor_tensor(
                vala[:], ygm[:], 0.0, t1a[:], op0=OP.is_gt, op1=OP.mult
            )
            mval = acc_pool.tile([128, PB], F32, tag="mval")
            nc.vector.tensor_reduce(
                mval[:], vala[:].rearrange("p (n l) -> p n l", l=L),
                axis=AX, op=OP.max,
            )

            # ---- epilogue: three parallel chains on [128, PB]
            def sigmoid_exp(name, src, scale, bias):
                """sigmoid(scale*u + bias) via the exp/ln table set:
                1/(1 + exp(-(scale*u + bias)))."""
                e = acc_pool.tile([128, PB], F32, tag=f"e_{name}", name=f"e_{name}")
                nc.scalar.activation(e[:], src, ACT.Exp, scale=-scale, bias=bias)
                a = acc_pool.tile([128, PB], F32, tag=f"a_{name}", name=f"a_{name}")
                nc.vector.tensor_scalar_add(a[:], e[:], 1.0)
                o = acc_pool.tile([128, PB], F32, tag=f"s_{name}", name=f"s_{name}")
                nc.vector.reciprocal(o[:], a[:])
                return o

            # W chain (rank_wl): x1r = mval - 32*trunc(mval/32) = gmax+8
            spi = acc_pool.tile([128, PB], I32, tag="spi")
            nc.vector.tensor_scalar_mul(spi[:], mval[:], 1.0 / 32.0)
            x1r = acc_pool.tile([128, PB], F32, tag="x1r")
            nc.vector.scalar_tensor_tensor(
                x1r[:], spi[:], -32.0, mval[:], op0=OP.mult, op1=OP.add
            )
            if tanh_epi:
                # sig chains via tanh (same table set as Exp):
                # sig(u) = (tanh(u/2)+1)/2, and the nested pair folds:
                # sig(10*s-4.5) with s=(t+1)/2  ->  (tanh(2.5*t+0.25)+1)/2
                t1 = acc_pool.tile([128, PB], F32, tag="t1w")
                nc.scalar.activation(t1[:], x1r[:], ACT.Tanh, scale=0.5, bias=bm4[:])
                t1b = acc_pool.tile([128, PB], F32, tag="t1b")
                nc.scalar.activation(t1b[:], t1[:], ACT.Tanh, scale=-2.5, bias=b025[:])
                swl = acc_pool.tile([128, PB], F32, tag="swl")
                nc.vector.tensor_scalar(
                    swl[:], t1b[:], 1.0, 0.5, op0=OP.add, op1=OP.mult
                )
            elif lse:
                x1s = sigmoid_exp("x1s", x1r[:], 1.0, b8[:])       # sig(x1r-8)
                swl = sigmoid_exp("swl", x1s[:], -10.0, bm55e[:])  # sig(5.5-10*x1s)
            else:
                x1s = acc_pool.tile([128, PB], F32, tag="x1s")
                nc.scalar.activation(
                    x1s[:], x1r[:], ACT.Sigmoid, scale=1.0, bias=bm8[:]
                )
                swl = acc_pool.tile([128, PB], F32, tag="swl")
                nc.scalar.activation(
                    swl[:], x1s[:], ACT.Sigmoid, scale=-10.0, bias=b55[:]
                )
            gwl = acc_pool.tile([128, PB], F32, tag="gwl")
            nc.vector.tensor_scalar(
                gwl[:], x1r[:], 8.0 + LOGIT_55, None, op0=OP.is_lt
            )
            rwl = acc_pool.tile([128, PB], F32, tag="rwl")
            nc.vector.scalar_tensor_tensor(
                rwl[:], gwl[:], 1.0, swl[:], op0=OP.add, op1=OP.mult
            )

            # O1 chain from mno
            if tanh_epi:
                t3 = acc_pool.tile([128, PB], F32, tag="t3o")
                nc.scalar.activation(t3[:], mno[:], ACT.Tanh, scale=0.5, bias=bz[:])
                t3b = acc_pool.tile([128, PB], F32, tag="t3b")
                nc.scalar.activation(t3b[:], t3[:], ACT.Tanh, scale=2.5, bias=b025[:])
                s1 = acc_pool.tile([128, PB], F32, tag="s1")
                nc.vector.tensor_scalar(
                    s1[:], t3b[:], 1.0, 0.5, op0=OP.add, op1=OP.mult
                )
            elif lse:
                s1a = sigmoid_exp("s1a", mno[:], 1.0, 0.0)
                s1 = sigmoid_exp("s1", s1a[:], 10.0, b45[:])       # sig(10*s1a-4.5)
            else:
                s1a = acc_pool.tile([128, PB], F32, tag="s1a")
                nc.scalar.activation(s1a[:], mno[:], ACT.Sigmoid)
                s1 = acc_pool.tile([128, PB], F32, tag="s1")
                nc.scalar.activation(
                    s1[:], s1a[:], ACT.Sigmoid, scale=10.0, bias=bm45[:]
                )
            g1 = acc_pool.tile([128, PB], F32, tag="g1")
            nc.vector.tensor_scalar(g1[:], mno[:], -LOGIT_55, None, op0=OP.is_gt)
            r1 = acc_pool.tile([128, PB], F32, tag="r1")
            nc.vector.scalar_tensor_tensor(
                r1[:], g1[:], 1.0, s1[:], op0=OP.add, op1=OP.mult
            )

            # O2 chain from the wrong-col stat (v3: w is the max; v5: w is the
            # exp-sum and lw = ln(w) carries (K*max - C + ln neff))
            if tanh_epi:
                # lg = log2(wsum) via the classic bit-hack (err ~0.04 in log2
                # => ~0.0013 in w); then both sigmoids as folded tanh ops:
                # s2a=(tanh((lg*ln2+C)/(2K))+1)/2, s2=(tanh(2.5*t+0.25)+1)/2
                lg = acc_pool.tile([128, PB], F32, tag="lg")
                nc.vector.tensor_scalar(
                    lg[:], w[:].bitcast(I32), 2.0 ** -23, BITHACK_BIAS,
                    op0=OP.mult, op1=OP.add,
                )
                t2 = acc_pool.tile([128, PB], F32, tag="t2o")
                nc.scalar.activation(
                    t2[:], lg[:], ACT.Tanh,
                    scale=LN2 / (2 * K_LSE), bias=bC2K[:],
                )
                t2b = acc_pool.tile([128, PB], F32, tag="t2b")
                nc.scalar.activation(t2b[:], t2[:], ACT.Tanh, scale=2.5, bias=b025[:])
                s2 = acc_pool.tile([128, PB], F32, tag="s2")
                nc.vector.tensor_scalar(
                    s2[:], t2b[:], 1.0, 0.5, op0=OP.add, op1=OP.mult
                )
                g2 = acc_pool.tile([128, PB], F32, tag="g2")
                nc.vector.tensor_scalar(
                    g2[:], lg[:], (-K_LSE * LOGIT_55 - C_LSE) / LN2, None,
                    op0=OP.is_gt,
                )
            elif lse:
                lw = acc_pool.tile([128, PB], F32, tag="lw")
                nc.scalar.activation(lw[:], w[:], ACT.Ln)
                # s2a = sig((lw+C)/K) = 1/(1+exp(-lw/K - C/K))
                s2a = sigmoid_exp("s2a", lw[:], 1.0 / K_LSE, bmCk[:])
                s2 = sigmoid_exp("s2", s2a[:], 10.0, b45[:])
                g2 = acc_pool.tile([128, PB], F32, tag="g2")
                nc.vector.tensor_scalar(
                    g2[:], lw[:], -K_LSE * LOGIT_55 - C_LSE, None, op0=OP.is_gt
                )
            else:
                s2a = acc_pool.tile([128, PB], F32, tag="s2a")
                nc.scalar.activation(s2a[:], w[:], ACT.Sigmoid)
                s2 = acc_pool.tile([128, PB], F32, tag="s2")
                nc.scalar.activation(
                    s2[:], s2a[:], ACT.Sigmoid, scale=10.0, bias=bm45[:]
                )
                g2 = acc_pool.tile([128, PB], F32, tag="g2")
                nc.vector.tensor_scalar(g2[:], w[:], -LOGIT_55, None, op0=OP.is_gt)
            r2 = acc_pool.tile([128, PB], F32, tag="r2")
            nc.vector.scalar_tensor_tensor(
                r2[:], g2[:], 1.0, s2[:], op0=OP.add, op1=OP.mult
            )

            # merge: loss = has_wl ? rwl : 0.5*(rank1 + rank2)
            ro = acc_pool.tile([128, PB], F32, tag="ro")
            nc.vector.tensor_add(ro[:], r1[:], r2[:])
            nc.vector.tensor_scalar_mul(ro[:], ro[:], 0.5)
            hwl = acc_pool.tile([128, PB], I32, tag="hwl")
            nc.vector.tensor_scalar(hwl[:], mval[:], 16.0, None, op0=OP.is_gt)
            loss = acc_pool.tile([128, PB], F32, tag="loss")
            nc.vector.select(loss[:], hwl[:], rwl[:], ro[:])

            # sum: PE reduces partitions to [1, PB]; ACT copy accumulates to [1,1]
            if True:
                ps = psum_pool.tile([1, PB], F32)
                nc.tensor.matmul(ps[:], ones[:], loss[:], start=True, stop=True)
                res = acc_pool.tile([1, 1], F32, tag="res")
                scr4 = acc_pool.tile([1, PB], F32, tag="scr4")
                nc.scalar.activation(scr4[:], ps[:], ACT.Copy, accum_out=res[:])
            nc.sync.dma_start(out_ext[:, :], res[:])

    nc.finalize()
    return nc


DEFAULT_VARIANT = "v3"

_NC_CACHE = {}


def _get_nc(variant=None):
    if variant is None:
        variant = DEFAULT_VARIANT
    if variant not in _NC_CACHE:
        _NC_CACHE[variant] = build_nc(variant=variant)
    return _NC_CACHE[variant]


_BF16NP = mybir.dt.np(BF16)


def _col_reorder(a):
    """[R, 1000] -> A|B halves: each group's cols 0:25 then 25:50, contiguous."""
    r = a.reshape(-1, L, 2, G // 2)
    return np.concatenate(
        [np.ascontiguousarray(r[:, :, 0, :]).reshape(-1, L * (G // 2)),
         np.ascontiguousarray(r[:, :, 1, :]).reshape(-1, L * (G // 2))],
        axis=1,
    )


def _dev_layout(a, W):
    """[RPC, W] -> [128, PB*W] block-major partition layout."""
    return np.ascontiguousarray(
        a.reshape(PB, 128, W).transpose(1, 0, 2).reshape(128, PB * W)
    )


_POW25 = (1 << np.arange(G // 2, dtype=np.int64)).astype(np.int64)


def make_in_maps(x, y, y_neg, variant="v3"):
    m_np = _BF16NP if ("w" in variant or variant.startswith("v5") or variant.startswith("v6")) else np.int8
    in_maps = []
    for i in range(NCORES):
        r0 = i * RPC
        xs = _col_reorder(np.asarray(x[r0 : r0 + RPC, :CU])).astype(_BF16NP)
        m = np.where(
            _col_reorder(np.asarray(y_neg[r0 : r0 + RPC, :CU])) != 0, 0, MNEG
        ).astype(m_np)
        bits = (np.asarray(y[r0 : r0 + RPC, :CU]) != 0).reshape(RPC, L, 2, G // 2)
        words = (bits * _POW25).sum(axis=-1).astype(np.int32)  # [RPC, L, 2] < 2^25
        in_maps.append({
            "x": np.ascontiguousarray(xs),
            "m": _dev_layout(m, CU),
            "y_p": _dev_layout(words.reshape(RPC, L * 2), L * 2),
        })
    return in_maps


def kernel(x, y, y_neg, wl_masks=None, **_):
    x = np.asarray(x)
    y = np.asarray(y)
    y_neg = np.asarray(y_neg)
    assert x.shape == (B, C), x.shape
    nc = _get_nc()
    in_maps = make_in_maps(x, y, y_neg, variant=DEFAULT_VARIANT)
    res = run_bass_kernel_spmd(nc, in_maps, core_ids=list(range(NCORES)))
    total = np.float32(0.0)
    for r in res.results:
        total += np.float32(r["out"].reshape(-1)[0])
    return np.float32(total)


# revision 32
# speedup vs baseline: 1.0453x; 1.0453x over previous
"""Trainium2 Bass kernel for nn_AsymmetricLossCustomPriorityRankNewNegOne.

Pure data parallel across 8 NeuronCores: core i takes rows [i*512, (i+1)*512);
each core reduces its rows to a partial scalar on-device and the host adds the
8 partials (the trivial all-reduce).

Only columns [0, 1000) of the 9605-wide inputs are ever used (the whitelist
masks cover exactly those); sigmoid is monotone so all masked maxes run on raw
x and sigmoid is applied to per-row scalars only.

v2 structure (per 128-row block, 4 blocks per core):
  * x ships as bf16 with each group's columns split into two contiguous
    25-col halves (A|B layout, host-side per-tensor relayout), so a 2x-mode
    DVE tensor_tensor max folds 1000 -> 500 before the s=25 group reduce.
    The halving runs on the (otherwise idle) GpSimd engine.
  * y_neg ships as an int8 additive mask m in {0,-64}; one SWDGE cast-DMA
    expands it to bf16 on the way into SBUF. The whole wrong-column pass is
    ONE fused DVE tensor_tensor_reduce: accum = max(x + m) per row.
    (Masked columns sit at x-64 < any unmasked x, and an empty wrong-set
    degenerates to sigmoid(-59) ~ 0, matching the reference's NEG_INF.)
  * y ships as packbits words (two 25-bit int32 words per 50-col group,
    lossless re-encoding); the presence OR is an s=2 max reduce on device.
  * first-present-group selection via priority encoding
    val[l] = present[l] * ((L-l)*32 + gmax[l] + 8); max_l val picks the
    lowest present l and mod(mval,32)-8 recovers its group max.
  * epilogue is three short parallel chains (rank_wl / rank_other halves)
    with compare thresholds hoisted to pre-sigmoid space (logit(0.55) etc.)
    so the ACT sigmoids and DVE compares run concurrently; the partition sum
    runs on the PE with the final accumulation in the ACT copy.
"""

import numpy as np
import sys
from contextlib import ExitStack

sys.path.insert(0, "/opt/trn_rl_repo")

import concourse.bass as bass
import concourse.bacc as bacc
import concourse.mybir as mybir
import concourse.tile as tile
from concourse.bass_utils import run_bass_kernel_spmd

B, C = 4096, 9605
L, G = 20, 50
CU = L * G          # 1000 used columns
H = CU // 2         # 500 (A|B halves)
NCORES = 8
RPC = B // NCORES   # 512 rows per core
PB = RPC // 128     # 4 partition blocks of 128 rows

F32 = mybir.dt.float32
I32 = mybir.dt.int32
I8 = mybir.dt.int8
BF16 = mybir.dt.bfloat16
AX = mybir.AxisListType.X
OP = mybir.AluOpType
ACT = mybir.ActivationFunctionType

LOGIT_55 = 0.2006707  # logit(0.55); logit(0.45) = -LOGIT_55
MNEG = -64.0          # additive mask for non-wrong columns


K_LSE = 22.0   # exp sharpness for the ACT-engine wrong-col log-sum-exp
C_LSE = 47.0   # exp offset: max arg ~ 22*5.6-47 = 76 keeps fp32 sums < 1.3e36
LN2 = 0.6931471805599453
BITHACK_BIAS = -126.94269504  # log2(u) ~ float(bits(u))*2^-23 + BITHACK_BIAS


def build_nc(reps=1, loop_n=None, variant="v3"):
    lse = variant.startswith("v5") or (
        variant.startswith("v6") and not variant.startswith("v6x")
    )
    tanh_epi = variant.startswith("v6")
    m_bf16_wire = "w" in variant or lse
    n_pool_adds = 0
    if "p1" in variant:
        n_pool_adds = 1
    elif "p2" in variant:
        n_pool_adds = 2
    nc = bacc.Bacc()
    x_ext = nc.declare_dram_parameter("x", [RPC, CU], BF16, isOutput=False)
    m_ext = nc.declare_dram_parameter(
        "m", [128, PB * CU], BF16 if m_bf16_wire else I8, isOutput=False
    )
    yp_ext = nc.declare_dram_parameter("y_p", [128, PB * L * 2], I32, isOutput=False)
    out_ext = nc.declare_dram_parameter("out", [1, 1], F32, isOutput=True)

    # which blocks route the wrong-pass add through Pool (Pool TT supports
    # add/mult only — max must stay on DVE)
    pool_add = tuple(n >= PB - n_pool_adds for n in range(PB))

    with ExitStack() as ctx:
        tc = ctx.enter_context(tile.TileContext(nc))
        const_pool = ctx.enter_context(tc.tile_pool(name="const", bufs=1))
        in_pool = ctx.enter_context(tc.tile_pool(name="inp", bufs=3))
        mid_pool = ctx.enter_context(tc.tile_pool(name="mid", bufs=3))
        acc_pool = ctx.enter_context(tc.tile_pool(name="acc", bufs=2))
        psum_pool = ctx.enter_context(tc.tile_pool(name="psum", bufs=1, space="PSUM"))

        # constants
        prio80 = const_pool.tile([128, PB * L], F32)
        nc.gpsimd.iota(
            prio80[:], pattern=[[0, PB], [-32, L]], base=int(L * 32 + 8),
            channel_multiplier=0, allow_small_or_imprecise_dtypes=True,
        )
        ones = const_pool.tile([128, 1], F32)
        nc.vector.memset(ones[:], 1.0)
        # dummy activation first: pins the ACT table set used by the kernel
        # (sigmoid set for v3, ln+exp set for v5) so no mid-kernel
        # LoadActFuncSet reload occurs
        actwarm = const_pool.tile([1, 1], F32)
        if tanh_epi:
            nc.scalar.activation(actwarm[:], ones[0:1, 0:1], ACT.Exp)
        elif lse:
            nc.scalar.activation(actwarm[:], ones[0:1, 0:1], ACT.Ln)
            nc.scalar.activation(actwarm[:], ones[0:1, 0:1], ACT.Exp)
        else:
            nc.scalar.activation(actwarm[:], ones[0:1, 0:1], ACT.Sigmoid)
        bm8 = const_pool.tile([128, 1], F32)
        nc.vector.memset(bm8[:], -8.0)
        b55 = const_pool.tile([128, 1], F32)
        nc.vector.memset(b55[:], 5.5)
        bm45 = const_pool.tile([128, 1], F32)
        nc.vector.memset(bm45[:], -4.5)
        b8 = const_pool.tile([128, 1], F32)
        nc.vector.memset(b8[:], 8.0)
        bm55e = const_pool.tile([128, 1], F32)
        nc.vector.memset(bm55e[:], -5.5)
        b45 = const_pool.tile([128, 1], F32)
        nc.vector.memset(b45[:], 4.5)
        bmC = const_pool.tile([128, 1], F32)
        nc.vector.memset(bmC[:], -C_LSE)
        bmCk = const_pool.tile([128, 1], F32)
        nc.vector.memset(bmCk[:], -C_LSE / K_LSE)
        bm4 = const_pool.tile([128, 1], F32)
        nc.vector.memset(bm4[:], -4.0)
        b025 = const_pool.tile([128, 1], F32)
        nc.vector.memset(b025[:], 0.25)
        bC2K = const_pool.tile([128, 1], F32)
        nc.vector.memset(bC2K[:], C_LSE / (2 * K_LSE))
        bz = const_pool.tile([128, 1], F32)
        nc.vector.memset(bz[:], 0.0)

        import contextlib
        loop_cm = tc.For_i(0, loop_n, 1) if loop_n else contextlib.nullcontext()
        with loop_cm:
          for _rep in range(reps):
            # ---- tiles
            xts = [
                in_pool.tile([128, CU], BF16, tag=f"xt{n}", name=f"xt{n}")
                for n in range(PB)
            ]
            mt = acc_pool.tile([128, PB * CU], BF16)
            ypt = acc_pool.tile([128, PB * L * 2], I32)
            gm_all = acc_pool.tile([128, PB * L], F32)
            ygm = acc_pool.tile([128, PB * L], F32)
            vala = acc_pool.tile([128, PB * L], F32)
            t1a = acc_pool.tile([128, PB * L], F32)
            w = acc_pool.tile([128, PB], F32)

            # ---- DMAs: x block 0 first (DVE's first dependency); per-block
            # int8 mask slices interleave with the x blocks so each block's
            # operands land (and its ACT i8->bf16 cast runs) just in time.
            mi = acc_pool.tile([128, PB * CU], I8, name="mi")
            mdst = mt if m_bf16_wire else mi
            nc.sync.dma_start(xts[0][:], x_ext[bass.ts(0, 128), :])
            nc.sync.dma_start(mdst[:, bass.ts(0, CU)], m_ext[:, bass.ts(0, CU)])
            for n in range(1, PB):
                nc.sync.dma_start(xts[n][:], x_ext[bass.ts(n, 128), :])
                nc.sync.dma_start(mdst[:, bass.ts(n, CU)], m_ext[:, bass.ts(n, CU)])
            nc.sync.dma_start(ypt[:], yp_ext[:])

            # mask expansion i8 -> bf16 on the (otherwise idle) ACT engine
            if not m_bf16_wire:
                for n in range(PB):
                    if not pool_add[n]:
                        nc.scalar.activation(
                            mt[:, bass.ts(n, CU)], mi[:, bass.ts(n, CU)], ACT.Copy
                        )

            # ---- presence OR: one s=2 max reduce over the packed words
            nc.vector.tensor_reduce(
                ygm[:], ypt[:].rearrange("p (m s) -> p m s", s=2),
                axis=AX, op=OP.max,
            )

            # ---- per-block: wrong pass (add + halve + reduce) and
            # group-max pass (halve + s=25 reduce), all 2x-mode where possible
            for n in range(PB):
                xt = xts[n]
                if pool_add[n]:
                    # Pool computes S = x + m (mixed dtypes, software convert);
                    # DVE halves + reduces it
                    sp = mid_pool.tile([128, CU], F32, tag=f"sp{n % 2}")
                    nc.gpsimd.tensor_tensor(
                        sp[:], xt[:], mi[:, bass.ts(n, CU)], op=OP.add
                    )
                    sh = mid_pool.tile([128, H], F32, tag=f"shp{n % 2}")
                    nc.vector.tensor_tensor(sh[:], sp[:, :H], sp[:, H:], op=OP.max)
                    nc.vector.tensor_reduce(
                        w[:, n : n + 1], sh[:], axis=AX, op=OP.max
                    )
                elif lse:
                    scr = mid_pool.tile([128, CU], BF16, tag=f"scr{n % 2}")
                    nc.vector.tensor_tensor(
                        scr[:], xt[:], mt[:, bass.ts(n, CU)], op=OP.add
                    )
                    # fused exp + row-sum on ACT: wsum = sum exp(K*S - C);
                    # masked cols underflow to exactly 0
                    escr = mid_pool.tile([128, CU], BF16, tag=f"escr{n % 2}")
                    nc.scalar.activation(
                        escr[:], scr[:], ACT.Exp, scale=K_LSE, bias=bmC[:],
                        accum_out=w[:, n : n + 1],
                    )
                else:
                    scr = mid_pool.tile([128, CU], BF16, tag=f"scr{n % 2}")
                    nc.vector.tensor_tensor(
                        scr[:], xt[:], mt[:, bass.ts(n, CU)], op=OP.add
                    )
                    sh = mid_pool.tile([128, H], BF16, tag=f"sh{n % 2}")
                    nc.vector.tensor_tensor(sh[:], scr[:, :H], scr[:, H:], op=OP.max)
                    nc.vector.tensor_reduce(
                        w[:, n : n + 1], sh[:], axis=AX, op=OP.max
                    )
                xh = mid_pool.tile([128, H], BF16, tag=f"xhv{n % 2}")
                nc.vector.tensor_tensor(xh[:], xt[:, :H], xt[:, H:], op=OP.max)
                nc.vector.tensor_reduce(
                    gm_all[:, bass.ts(n, L)],
                    xh[:].rearrange("p (g s) -> p g s", s=G // 2),
                    axis=AX, op=OP.max,
                )
                # priority add off the critical path (Pool)
                nc.gpsimd.tensor_tensor(
                    t1a[:, bass.ts(n, L)], gm_all[:, bass.ts(n, L)],
                    prio80[:, bass.ts(n, L)], op=OP.add,
                )

            # ---- batched priority-encode + row stats
            mno = acc_pool.tile([128, PB], F32, tag="mno")
            nc.vector.tensor_reduce(
                mno[:], gm_all[:].rearrange("p (n l) -> p n l", l=L),
                axis=AX, op=OP.max,
            )
            nc.vector.scalar_tensor_tensor(
                vala[:], ygm[:], 0.0, t1a[:], op0=OP.is_gt, op1=OP.mult
            )
            mval = acc_pool.tile([128, PB], F32, tag="mval")
            nc.vector.tensor_reduce(
                mval[:], vala[:].rearrange("p (n l) -> p n l", l=L),
                axis=AX, op=OP.max,
            )

            # ---- epilogue: three parallel chains on [128, PB]
            def sigmoid_exp(name, src, scale, bias):
                """sigmoid(scale*u + bias) via the exp/ln table set:
                1/(1 + exp(-(scale*u + bias)))."""
                e = acc_pool.tile([128, PB], F32, tag=f"e_{name}", name=f"e_{name}")
                nc.scalar.activation(e[:], src, ACT.Exp, scale=-scale, bias=bias)
                a = acc_pool.tile([128, PB], F32, tag=f"a_{name}", name=f"a_{name}")
                nc.vector.tensor_scalar_add(a[:], e[:], 1.0)
                o = acc_pool.tile([128, PB], F32, tag=f"s_{name}", name=f"s_{name}")
                nc.vector.reciprocal(o[:], a[:])
                return o

            # W chain (rank_wl): x1r = mval - 32*trunc(mval/32) = gmax+8
            spi = acc_pool.tile([128, PB], I32, tag="spi")
            nc.vector.tensor_scalar_mul(spi[:], mval[:], 1.0 / 32.0)
            x1r = acc_pool.tile([128, PB], F32, tag="x1r")
            nc.vector.scalar_tensor_tensor(
                x1r[:], spi[:], -32.0, mval[:], op0=OP.mult, op1=OP.add
            )
            if tanh_epi:
                # sig chains via tanh (same table set as Exp):
                # sig(u) = (tanh(u/2)+1)/2, and the nested pair folds:
                # sig(10*s-4.5) with s=(t+1)/2  ->  (tanh(2.5*t+0.25)+1)/2
                t1 = acc_pool.tile([128, PB], F32, tag="t1w")
                nc.scalar.activation(t1[:], x1r[:], ACT.Tanh, scale=0.5, bias=bm4[:])
                t1b = acc_pool.tile([128, PB], F32, tag="t1b")
                nc.scalar.activation(t1b[:], t1[:], ACT.Tanh, scale=-2.5, bias=b025[:])
                swl = acc_pool.tile([128, PB], F32, tag="swl")
                nc.vector.tensor_scalar(
                    swl[:], t1b[:], 1.0, 0.5, op0=OP.add, op1=OP.mult
                )
            elif lse:
                x1s = sigmoid_exp("x1s", x1r[:], 1.0, b8[:])       # sig(x1r-8)
                swl = sigmoid_exp("swl", x1s[:], -10.0, bm55e[:])  # sig(5.5-10*x1s)
            else:
                x1s = acc_pool.tile([128, PB], F32, tag="x1s")
                nc.scalar.activation(
                    x1s[:], x1r[:], ACT.Sigmoid, scale=1.0, bias=bm8[:]
                )
                swl = acc_pool.tile([128, PB], F32, tag="swl")
                nc.scalar.activation(
                    swl[:], x1s[:], ACT.Sigmoid, scale=-10.0, bias=b55[:]
                )
            gwl = acc_pool.tile([128, PB], F32, tag="gwl")
            nc.vector.tensor_scalar(
                gwl[:], x1r[:], 8.0 + LOGIT_55, None, op0=OP.is_lt
            )
            rwl = acc_pool.tile([128, PB], F32, tag="rwl")
            nc.vector.scalar_tensor_tensor(
                rwl[:], gwl[:], 1.0, swl[:], op0=OP.add, op1=OP.mult
            )

            # O1 chain from mno
            if tanh_epi:
                t3 = acc_pool.tile([128, PB], F32, tag="t3o")
                nc.scalar.activation(t3[:], mno[:], ACT.Tanh, scale=0.5, bias=bz[:])
                t3b = acc_pool.tile([128, PB], F32, tag="t3b")
                nc.scalar.activation(t3b[:], t3[:], ACT.Tanh, scale=2.5, bias=b025[:])
                s1 = acc_pool.tile([128, PB], F32, tag="s1")
                nc.vector.tensor_scalar(
                    s1[:], t3b[:], 1.0, 0.5, op0=OP.add, op1=OP.mult
                )
            elif lse:
                s1a = sigmoid_exp("s1a", mno[:], 1.0, 0.0)
                s1 = sigmoid_exp("s1", s1a[:], 10.0, b45[:])       # sig(10*s1a-4.5)
            else:
                s1a = acc_pool.tile([128, PB], F32, tag="s1a")
                nc.scalar.activation(s1a[:], mno[:], ACT.Sigmoid)
                s1 = acc_pool.tile([128, PB], F32, tag="s1")
                nc.scalar.activation(
                    s1[:], s1a[:], ACT.Sigmoid, scale=10.0, bias=bm45[:]
                )
            g1 = acc_pool.tile([128, PB], F32, tag="g1")
            nc.vector.tensor_scalar(g1[:], mno[:], -LOGIT_55, None, op0=OP.is_gt)
            r1 = acc_pool.tile([128, PB], F32, tag="r1")
            nc.vector.scalar_tensor_tensor(
                r1[:], g1[:], 1.0, s1[:], op0=OP.add, op1=OP.mult
            )

            # O2 chain from the wrong-col stat (v3: w is the max; v5: w is the
            # exp-sum and lw = ln(w) carries (K*max - C + ln neff))
            if tanh_epi and lse:
                # lg = log2(wsum) via the classic bit-hack (err ~0.04 in log2
                # => ~0.0013 in w); then both sigmoids as folded tanh ops:
                # s2a=(tanh((lg*ln2+C)/(2K))+1)/2, s2=(tanh(2.5*t+0.25)+1)/2
                lg = acc_pool.tile([128, PB], F32, tag="lg")
                nc.vector.tensor_scalar(
                    lg[:], w[:].bitcast(I32), 2.0 ** -23, BITHACK_BIAS,
                    op0=OP.mult, op1=OP.add,
                )
                t2 = acc_pool.tile([128, PB], F32, tag="t2o")
                nc.scalar.activation(
                    t2[:], lg[:], ACT.Tanh,
                    scale=LN2 / (2 * K_LSE), bias=bC2K[:],
                )
                t2b = acc_pool.tile([128, PB], F32, tag="t2b")
                nc.scalar.activation(t2b[:], t2[:], ACT.Tanh, scale=2.5, bias=b025[:])
                s2 = acc_pool.tile([128, PB], F32, tag="s2")
                nc.vector.tensor_scalar(
                    s2[:], t2b[:], 1.0, 0.5, op0=OP.add, op1=OP.mult
                )
                g2 = acc_pool.tile([128, PB], F32, tag="g2")
                nc.vector.tensor_scalar(
                    g2[:], lg[:], (-K_LSE * LOGIT_55 - C_LSE) / LN2, None,
                    op0=OP.is_gt,
                )
            elif tanh_epi:
                t2 = acc_pool.tile([128, PB], F32, tag="t2o")
                nc.scalar.activation(t2[:], w[:], ACT.Tanh, scale=0.5, bias=bz[:])
                t2b = acc_pool.tile([128, PB], F32, tag="t2b")
                nc.scalar.activation(t2b[:], t2[:], ACT.Tanh, scale=2.5, bias=b025[:])
                s2 = acc_pool.tile([128, PB], F32, tag="s2")
                nc.vector.tensor_scalar(
                    s2[:], t2b[:], 1.0, 0.5, op0=OP.add, op1=OP.mult
                )
                g2 = acc_pool.tile([128, PB], F32, tag="g2")
                nc.vector.tensor_scalar(g2[:], w[:], -LOGIT_55, None, op0=OP.is_gt)
            elif lse:
                lw = acc_pool.tile([128, PB], F32, tag="lw")
                nc.scalar.activation(lw[:], w[:], ACT.Ln)
                # s2a = sig((lw+C)/K) = 1/(1+exp(-lw/K - C/K))
                s2a = sigmoid_exp("s2a", lw[:], 1.0 / K_LSE, bmCk[:])
                s2 = sigmoid_exp("s2", s2a[:], 10.0, b45[:])
                g2 = acc_pool.tile([128, PB], F32, tag="g2")
                nc.vector.tensor_scalar(
                    g2[:], lw[:], -K_LSE * LOGIT_55 - C_LSE, None, op0=OP.is_gt
                )
            else:
                s2a = acc_pool.tile([128, PB], F32, tag="s2a")
                nc.scalar.activation(s2a[:], w[:], ACT.Sigmoid)
                s2 = acc_pool.tile([128, PB], F32, tag="s2")
                nc.scalar.activation(
                    s2[:], s2a[:], ACT.Sigmoid, scale=10.0, bias=bm45[:]
                )
                g2 = acc_pool.tile([128, PB], F32, tag="g2")
                nc.vector.tensor_scalar(g2[:], w[:], -LOGIT_55, None, op0=OP.is_gt)
            r2 = acc_pool.tile([128, PB], F32, tag="r2")
            nc.vector.scalar_tensor_tensor(
                r2[:], g2[:], 1.0, s2[:], op0=OP.add, op1=OP.mult
            )

            # merge: loss = has_wl ? rwl : 0.5*(rank1 + rank2)
            ro = acc_pool.tile([128, PB], F32, tag="ro")
            nc.vector.tensor_add(ro[:], r1[:], r2[:])
            nc.vector.tensor_scalar_mul(ro[:], ro[:], 0.5)
            hwl = acc_pool.tile([128, PB], I32, tag="hwl")
            nc.vector.tensor_scalar(hwl[:], mval[:], 16.0, None, op0=OP.is_gt)
            loss = acc_pool.tile([128, PB], F32, tag="loss")
            nc.vector.select(loss[:], hwl[:], rwl[:], ro[:])

            # sum: PE reduces partitions to [1, PB]; ACT copy accumulates to [1,1]
            if True:
                ps = psum_pool.tile([1, PB], F32)
                nc.tensor.matmul(ps[:], ones[:], loss[:], start=True, stop=True)
                res = acc_pool.tile([1, 1], F32, tag="res")
                scr4 = acc_pool.tile([1, PB], F32, tag="scr4")
                nc.scalar.activation(scr4[:], ps[:], ACT.Copy, accum_out=res[:])
            nc.sync.dma_start(out_ext[:, :], res[:])

    nc.finalize()
    return nc


DEFAULT_VARIANT = "v3"

_NC_CACHE = {}


def _get_nc(variant=None):
    if variant is None:
        variant = DEFAULT_VARIANT
    if variant not in _NC_CACHE:
        _NC_CACHE[variant] = build_nc(variant=variant)
    return _NC_CACHE[variant]


_BF16NP = mybir.dt.np(BF16)


def _col_reorder(a):
    """[R, 1000] -> A|B halves: each group's cols 0:25 then 25:50, contiguous."""
    r = a.reshape(-1, L, 2, G // 2)
    return np.concatenate(
        [np.ascontiguousarray(r[:, :, 0, :]).reshape(-1, L * (G // 2)),
         np.ascontiguousarray(r[:, :, 1, :]).reshape(-1, L * (G // 2))],
        axis=1,
    )


def _dev_layout(a, W):
    """[RPC, W] -> [128, PB*W] block-major partition layout."""
    return np.ascontiguousarray(
        a.reshape(PB, 128, W).transpose(1, 0, 2).reshape(128, PB * W)
    )


_POW25 = (1 << np.arange(G // 2, dtype=np.int64)).astype(np.int64)


def make_in_maps(x, y, y_neg, variant="v3"):
    m_np = _BF16NP if ("w" in variant or variant.startswith("v5") or (variant.startswith("v6") and not variant.startswith("v6x"))) else np.int8
    in_maps = []
    for i in range(NCORES):
        r0 = i * RPC
        xs = _col_reorder(np.asarray(x[r0 : r0 + RPC, :CU])).astype(_BF16NP)
        m = np.where(
            _col_reorder(np.asarray(y_neg[r0 : r0 + RPC, :CU])) != 0, 0, MNEG
        ).astype(m_np)
        bits = (np.asarray(y[r0 : r0 + RPC, :CU]) != 0).reshape(RPC, L, 2, G // 2)
        words = (bits * _POW25).sum(axis=-1).astype(np.int32)  # [RPC, L, 2] < 2^25
        in_maps.append({
            "x": np.ascontiguousarray(xs),
            "m": _dev_layout(m, CU),
            "y_p": _dev_layout(words.reshape(RPC, L * 2), L * 2),
        })
    return in_maps


def kernel(x, y, y_neg, wl_masks=None, **_):
    x = np.asarray(x)
    y = np.asarray(y)
    y_neg = np.asarray(y_neg)
    assert x.shape == (B, C), x.shape
    nc = _get_nc()
    in_maps = make_in_maps(x, y, y_neg, variant=DEFAULT_VARIANT)
    res = run_bass_kernel_spmd(nc, in_maps, core_ids=list(range(NCORES)))
    total = np.float32(0.0)
    for r in res.results:
        total += np.float32(r["out"].reshape(-1)[0])
    return np.float32(total)


# revision 39
# speedup vs baseline: 1.2060x; 1.1537x over previous
"""Trainium2 Bass kernel for nn_AsymmetricLossCustomPriorityRankNewNegOne.

Pure data parallel across 8 NeuronCores: core i takes rows [i*512, (i+1)*512);
each core reduces its rows to a partial scalar on-device and the host adds the
8 partials (the trivial all-reduce).

Only columns [0, 1000) of the 9605-wide inputs are ever used (the whitelist
masks cover exactly those); sigmoid is monotone so all masked maxes run on raw
x and sigmoid is applied to per-row scalars only.

v2 structure (per 128-row block, 4 blocks per core):
  * x ships as bf16 with each group's columns split into two contiguous
    25-col halves (A|B layout, host-side per-tensor relayout), so a 2x-mode
    DVE tensor_tensor max folds 1000 -> 500 before the s=25 group reduce.
    The halving runs on the (otherwise idle) GpSimd engine.
  * y_neg ships as an int8 additive mask m in {0,-64}; one SWDGE cast-DMA
    expands it to bf16 on the way into SBUF. The whole wrong-column pass is
    ONE fused DVE tensor_tensor_reduce: accum = max(x + m) per row.
    (Masked columns sit at x-64 < any unmasked x, and an empty wrong-set
    degenerates to sigmoid(-59) ~ 0, matching the reference's NEG_INF.)
  * y ships as packbits words (two 25-bit int32 words per 50-col group,
    lossless re-encoding); the presence OR is an s=2 max reduce on device.
  * first-present-group selection via priority encoding
    val[l] = present[l] * ((L-l)*32 + gmax[l] + 8); max_l val picks the
    lowest present l and mod(mval,32)-8 recovers its group max.
  * epilogue is three short parallel chains (rank_wl / rank_other halves)
    with compare thresholds hoisted to pre-sigmoid space (logit(0.55) etc.)
    so the ACT sigmoids and DVE compares run concurrently; the partition sum
    runs on the PE with the final accumulation in the ACT copy.
"""

import numpy as np
import sys
from contextlib import ExitStack

sys.path.insert(0, "/opt/trn_rl_repo")

import concourse.bass as bass
import concourse.bacc as bacc
import concourse.mybir as mybir
import concourse.tile as tile
from concourse.bass_utils import run_bass_kernel_spmd

B, C = 4096, 9605
L, G = 20, 50
CU = L * G          # 1000 used columns
H = CU // 2         # 500 (A|B halves)
NCORES = 8
RPC = B // NCORES   # 512 rows per core
PB = RPC // 128     # 4 partition blocks of 128 rows

F32 = mybir.dt.float32
I32 = mybir.dt.int32
I8 = mybir.dt.int8
BF16 = mybir.dt.bfloat16
AX = mybir.AxisListType.X
OP = mybir.AluOpType
ACT = mybir.ActivationFunctionType

LOGIT_55 = 0.2006707  # logit(0.55); logit(0.45) = -LOGIT_55
MNEG = -64.0          # additive mask for non-wrong columns


K_LSE = 22.0   # exp sharpness for the ACT-engine wrong-col log-sum-exp
C_LSE = 47.0   # exp offset: max arg ~ 22*5.6-47 = 76 keeps fp32 sums < 1.3e36
LN2 = 0.6931471805599453
BITHACK_BIAS = -126.94269504  # log2(u) ~ float(bits(u))*2^-23 + BITHACK_BIAS


def build_nc(reps=1, loop_n=None, variant="v3"):
    lse = variant.startswith("v5") or (
        variant.startswith("v6") and not variant.startswith("v6x")
    )
    tanh_epi = variant.startswith("v6")
    m_bf16_wire = "w" in variant or lse
    n_pool_adds = 0
    if "p1" in variant:
        n_pool_adds = 1
    elif "p2" in variant:
        n_pool_adds = 2
    nc = bacc.Bacc()
    x_ext = nc.declare_dram_parameter("x", [RPC, CU], BF16, isOutput=False)
    m_ext = nc.declare_dram_parameter(
        "m", [128, PB * CU], BF16 if m_bf16_wire else I8, isOutput=False
    )
    yp_ext = nc.declare_dram_parameter("y_p", [128, PB * L * 2], I32, isOutput=False)
    out_ext = nc.declare_dram_parameter("out", [1, PB], F32, isOutput=True)

    # which blocks route the wrong-pass add through Pool (Pool TT supports
    # add/mult only — max must stay on DVE)
    pool_add = tuple(n >= PB - n_pool_adds for n in range(PB))

    with ExitStack() as ctx:
        tc = ctx.enter_context(tile.TileContext(nc))
        const_pool = ctx.enter_context(tc.tile_pool(name="const", bufs=1))
        in_pool = ctx.enter_context(tc.tile_pool(name="inp", bufs=3))
        mid_pool = ctx.enter_context(tc.tile_pool(name="mid", bufs=3))
        acc_pool = ctx.enter_context(tc.tile_pool(name="acc", bufs=2))
        psum_pool = ctx.enter_context(tc.tile_pool(name="psum", bufs=1, space="PSUM"))

        # constants
        prio80 = const_pool.tile([128, PB * L], F32)
        nc.gpsimd.iota(
            prio80[:], pattern=[[0, PB], [-32, L]], base=int(L * 32 + 8),
            channel_multiplier=0, allow_small_or_imprecise_dtypes=True,
        )
        ones = const_pool.tile([128, 1], F32)
        nc.vector.memset(ones[:], 1.0)
        # dummy activation first: pins the ACT table set used by the kernel
        # (sigmoid set for v3, ln+exp set for v5) so no mid-kernel
        # LoadActFuncSet reload occurs
        actwarm = const_pool.tile([1, 1], F32)
        if tanh_epi:
            nc.scalar.activation(actwarm[:], ones[0:1, 0:1], ACT.Exp)
        elif lse:
            nc.scalar.activation(actwarm[:], ones[0:1, 0:1], ACT.Ln)
            nc.scalar.activation(actwarm[:], ones[0:1, 0:1], ACT.Exp)
        else:
            nc.scalar.activation(actwarm[:], ones[0:1, 0:1], ACT.Sigmoid)
        bm8 = const_pool.tile([128, 1], F32)
        nc.vector.memset(bm8[:], -8.0)
        b55 = const_pool.tile([128, 1], F32)
        nc.vector.memset(b55[:], 5.5)
        bm45 = const_pool.tile([128, 1], F32)
        nc.vector.memset(bm45[:], -4.5)
        b8 = const_pool.tile([128, 1], F32)
        nc.vector.memset(b8[:], 8.0)
        bm55e = const_pool.tile([128, 1], F32)
        nc.vector.memset(bm55e[:], -5.5)
        b45 = const_pool.tile([128, 1], F32)
        nc.vector.memset(b45[:], 4.5)
        bmC = const_pool.tile([128, 1], F32)
        nc.vector.memset(bmC[:], -C_LSE)
        bmCk = const_pool.tile([128, 1], F32)
        nc.vector.memset(bmCk[:], -C_LSE / K_LSE)
        bm4 = const_pool.tile([128, 1], F32)
        nc.vector.memset(bm4[:], -4.0)
        b025 = const_pool.tile([128, 1], F32)
        nc.vector.memset(b025[:], 0.25)
        bC2K = const_pool.tile([128, 1], F32)
        nc.vector.memset(bC2K[:], C_LSE / (2 * K_LSE))
        bz = const_pool.tile([128, 1], F32)
        nc.vector.memset(bz[:], 0.0)
        cmw = const_pool.tile([128, 3 * PB], F32)
        nc.vector.memset(cmw[:, : PB], 0.5)
        nc.vector.memset(cmw[:, PB :], 0.25)

        import contextlib
        loop_cm = tc.For_i(0, loop_n, 1) if loop_n else contextlib.nullcontext()
        with loop_cm:
          for _rep in range(reps):
            # ---- tiles
            xts = [
                in_pool.tile([128, CU], BF16, tag=f"xt{n}", name=f"xt{n}")
                for n in range(PB)
            ]
            mt = acc_pool.tile([128, PB * CU], BF16)
            ypt = acc_pool.tile([128, PB * L * 2], I32)
            gm_all = acc_pool.tile([128, PB * L], F32)
            ygm = acc_pool.tile([128, PB * L], F32)
            vala = acc_pool.tile([128, PB * L], F32)
            t1a = acc_pool.tile([128, PB * L], F32)
            w = acc_pool.tile([128, PB], F32)

            # ---- DMAs: x block 0 first (DVE's first dependency); per-block
            # int8 mask slices interleave with the x blocks so each block's
            # operands land (and its ACT i8->bf16 cast runs) just in time.
            mi = acc_pool.tile([128, PB * CU], I8, name="mi")
            mdst = mt if m_bf16_wire else mi
            nc.sync.dma_start(ypt[:], yp_ext[:])
            nc.sync.dma_start(xts[0][:], x_ext[bass.ts(0, 128), :])
            nc.sync.dma_start(mdst[:, bass.ts(0, CU)], m_ext[:, bass.ts(0, CU)])
            for n in range(1, PB):
                if n == PB - 1:
                    nc.sync.dma_start(
                        xts[n][:, :H], x_ext[bass.ts(n, 128), :H]
                    )
                    nc.sync.dma_start(
                        mdst[:, n * CU : n * CU + H], m_ext[:, n * CU : n * CU + H]
                    )
                    nc.sync.dma_start(
                        xts[n][:, H:], x_ext[bass.ts(n, 128), H:]
                    )
                    nc.sync.dma_start(
                        mdst[:, n * CU + H : (n + 1) * CU],
                        m_ext[:, n * CU + H : (n + 1) * CU],
                    )
                else:
                    nc.sync.dma_start(xts[n][:], x_ext[bass.ts(n, 128), :])
                    nc.sync.dma_start(
                        mdst[:, bass.ts(n, CU)], m_ext[:, bass.ts(n, CU)]
                    )

            # mask expansion i8 -> bf16 on the (otherwise idle) ACT engine
            if not m_bf16_wire:
                for n in range(PB):
                    if not pool_add[n]:
                        nc.scalar.activation(
                            mt[:, bass.ts(n, CU)], mi[:, bass.ts(n, CU)], ACT.Copy
                        )

            # ---- presence OR: one s=2 max reduce over the packed words
            nc.vector.tensor_reduce(
                ygm[:], ypt[:].rearrange("p (m s) -> p m s", s=2),
                axis=AX, op=OP.max,
            )
            # masked priorities: prem = present ? (L-l)*32+8 : 0 (early, so the
            # late vala is a single tensor_tensor add with the group maxes)
            prem = acc_pool.tile([128, PB * L], F32, name="prem")
            nc.vector.scalar_tensor_tensor(
                prem[:], ygm[:], 0.0, prio80[:], op0=OP.is_gt, op1=OP.mult
            )

            # ---- per-block: wrong pass (add + halve + reduce) and
            # group-max pass (halve + s=25 reduce), all 2x-mode where possible
            for n in range(PB):
                xt = xts[n]
                gmax_first = False
                if gmax_first:
                    xh = mid_pool.tile(
                        [128, H], BF16, tag=f"xhv{n % 2}", name="xh"
                    )
                    nc.vector.tensor_tensor(xh[:], xt[:, :H], xt[:, H:], op=OP.max)
                    nc.vector.tensor_reduce(
                        gm_all[:, bass.ts(n, L)],
                        xh[:].rearrange("p (g s) -> p g s", s=G // 2),
                        axis=AX, op=OP.max,
                    )
                if pool_add[n]:
                    # Pool computes S = x + m (mixed dtypes, software convert);
                    # DVE halves + reduces it
                    sp = mid_pool.tile([128, CU], F32, tag=f"sp{n % 2}")
                    nc.gpsimd.tensor_tensor(
                        sp[:], xt[:], mi[:, bass.ts(n, CU)], op=OP.add
                    )
                    sh = mid_pool.tile([128, H], F32, tag=f"shp{n % 2}")
                    nc.vector.tensor_tensor(sh[:], sp[:, :H], sp[:, H:], op=OP.max)
                    nc.vector.tensor_reduce(
                        w[:, n : n + 1], sh[:], axis=AX, op=OP.max
                    )
                elif lse:
                    scr = mid_pool.tile([128, CU], BF16, tag=f"scr{n % 2}")
                    if n == PB - 1:
                        # the last block's wrong-pass stays on DVE (a direct
                        # max): its ACT exp chain would gate the tail
                        nc.vector.tensor_tensor(
                            scr[:, :H], xt[:, :H], mt[:, n * CU : n * CU + H],
                            op=OP.add,
                        )
                        nc.vector.tensor_tensor(
                            scr[:, H:], xt[:, H:],
                            mt[:, n * CU + H : (n + 1) * CU], op=OP.add,
                        )
                        sh = mid_pool.tile([128, H], BF16, tag="sh3")
                        nc.vector.tensor_tensor(
                            sh[:], scr[:, :H], scr[:, H:], op=OP.max
                        )
                        nc.vector.tensor_reduce(
                            w[:, n : n + 1], sh[:], axis=AX, op=OP.max
                        )
                    else:
                        nc.vector.tensor_tensor(
                            scr[:], xt[:], mt[:, bass.ts(n, CU)], op=OP.add
                        )
                        # fused exp + row-sum on ACT: wsum = sum exp(K*S - C)
                        escr = mid_pool.tile([128, CU], BF16, tag=f"escr{n % 2}")
                        nc.scalar.activation(
                            escr[:], scr[:], ACT.Exp, scale=K_LSE, bias=bmC[:],
                            accum_out=w[:, n : n + 1],
                        )
                else:
                    scr = mid_pool.tile([128, CU], BF16, tag=f"scr{n % 2}")
                    nc.vector.tensor_tensor(
                        scr[:], xt[:], mt[:, bass.ts(n, CU)], op=OP.add
                    )
                    sh = mid_pool.tile([128, H], BF16, tag=f"sh{n % 2}")
                    nc.vector.tensor_tensor(sh[:], scr[:, :H], scr[:, H:], op=OP.max)
                    nc.vector.tensor_reduce(
                        w[:, n : n + 1], sh[:], axis=AX, op=OP.max
                    )
                if not gmax_first:
                    xh = mid_pool.tile(
                        [128, H], BF16, tag=f"xhv{n % 2}", name="xh"
                    )
                    nc.vector.tensor_tensor(xh[:], xt[:, :H], xt[:, H:], op=OP.max)
                    nc.vector.tensor_reduce(
                        gm_all[:, bass.ts(n, L)],
                        xh[:].rearrange("p (g s) -> p g s", s=G // 2),
                        axis=AX, op=OP.max,
                    )
                nc.vector.tensor_tensor(
                    vala[:, bass.ts(n, L)], prem[:, bass.ts(n, L)],
                    gm_all[:, bass.ts(n, L)], op=OP.add,
                )

            # ---- batched priority-encode + row stats
            mno = acc_pool.tile([128, PB], F32, tag="mno")
            nc.vector.tensor_reduce(
                mno[:], gm_all[:].rearrange("p (n l) -> p n l", l=L),
                axis=AX, op=OP.max,
            )
            mval = acc_pool.tile([128, PB], F32, tag="mval")
            nc.vector.tensor_reduce(
                mval[:], vala[:].rearrange("p (n l) -> p n l", l=L),
                axis=AX, op=OP.max,
            )

            # ---- epilogue (tanh path): the three rank chains batch into
            # one [128, 3*PB] tile. Each chain is a nested sigmoid pair
            # sig(10*sig(u)-4.5)-style, which folds to
            # t2 = tanh(2.5*tanh(u/2) + 0.25), rank = (g+1)*(t2+1)*c, and
            # after pre-scaling all three compares share ONE threshold.
            if tanh_epi:
                spi = acc_pool.tile([128, PB], I32, tag="spi")
                nc.vector.tensor_scalar_mul(spi[:], mval[:], 1.0 / 32.0)
                mv2n = acc_pool.tile([128, PB], F32, tag="mv2n")
                nc.vector.tensor_scalar(
                    mv2n[:], mval[:], -0.5, 4.0, op0=OP.mult, op1=OP.add
                )
                w2 = acc_pool.tile([128, 3 * PB], F32, tag="w2", name="w2")
                # slice0 = -(x1r/2 - 4) = 16*spi - mval/2 + 4  (sign absorbed
                # by the +2.5 second-stage scale since tanh is odd)
                nc.vector.scalar_tensor_tensor(
                    w2[:, :PB], spi[:], 16.0, mv2n[:], op0=OP.mult, op1=OP.add
                )
                # slice1 = mno/2
                nc.vector.tensor_scalar_mul(w2[:, PB : 2 * PB], mno[:], 0.5)
                # slice2 = w_true/2: bit-hack log2 of the exp-sums for the
                # first PB-1 cols, direct max/2 for the last
                nc.vector.tensor_scalar(
                    w2[:, 2 * PB : 3 * PB - 1], w[:, : PB - 1].bitcast(I32),
                    (2.0 ** -23) * LN2 / (2 * K_LSE),
                    (BITHACK_BIAS * LN2 + C_LSE) / (2 * K_LSE),
                    op0=OP.mult, op1=OP.add,
                )
                nc.vector.tensor_scalar_mul(
                    w2[:, 3 * PB - 1 : 3 * PB], w[:, PB - 1 : PB], 0.5
                )
                tb = acc_pool.tile([128, 3 * PB], F32, tag="tb", name="tb")
                nc.scalar.activation(tb[:], w2[:], ACT.Tanh, scale=1.0, bias=bz[:])
                t2b = acc_pool.tile([128, 3 * PB], F32, tag="t2b", name="t2b")
                nc.scalar.activation(t2b[:], tb[:], ACT.Tanh, scale=2.5, bias=b025[:])
                gb = acc_pool.tile([128, 3 * PB], F32, tag="gb", name="gb")
                nc.vector.tensor_scalar(
                    gb[:], w2[:], -LOGIT_55 / 2, None, op0=OP.is_gt
                )
                ub = acc_pool.tile([128, 3 * PB], F32, tag="ub", name="ub")
                nc.vector.scalar_tensor_tensor(
                    ub[:], t2b[:], 1.0, cmw[:], op0=OP.add, op1=OP.mult
                )
                rb = acc_pool.tile([128, 3 * PB], F32, tag="rb", name="rb")
                nc.vector.scalar_tensor_tensor(
                    rb[:], gb[:], 1.0, ub[:], op0=OP.add, op1=OP.mult
                )
                hwl = acc_pool.tile([128, PB], I32, tag="hwl")
                nc.vector.tensor_scalar(hwl[:], mval[:], 16.0, None, op0=OP.is_gt)
                loss = acc_pool.tile([128, PB], F32, tag="loss")
                nc.vector.tensor_add(loss[:], rb[:, PB : 2 * PB], rb[:, 2 * PB :])
                nc.vector.copy_predicated(loss[:], hwl[:], rb[:, :PB])
                ps = psum_pool.tile([1, PB], F32)
                nc.tensor.matmul(ps[:], ones[:], loss[:], start=True, stop=True)
                res = acc_pool.tile([1, PB], F32, tag="res")
                nc.vector.tensor_copy(res[:], ps[:])
                nc.sync.dma_start(out_ext[:, :], res[:])
                continue

            # ---- epilogue: three parallel chains on [128, PB]
            def sigmoid_exp(name, src, scale, bias):
                """sigmoid(scale*u + bias) via the exp/ln table set:
                1/(1 + exp(-(scale*u + bias)))."""
                e = acc_pool.tile([128, PB], F32, tag=f"e_{name}", name=f"e_{name}")
                nc.scalar.activation(e[:], src, ACT.Exp, scale=-scale, bias=bias)
                a = acc_pool.tile([128, PB], F32, tag=f"a_{name}", name=f"a_{name}")
                nc.vector.tensor_scalar_add(a[:], e[:], 1.0)
                o = acc_pool.tile([128, PB], F32, tag=f"s_{name}", name=f"s_{name}")
                nc.vector.reciprocal(o[:], a[:])
                return o

            # W chain (rank_wl): x1r = mval - 32*trunc(mval/32) = gmax+8
            spi = acc_pool.tile([128, PB], I32, tag="spi")
            nc.vector.tensor_scalar_mul(spi[:], mval[:], 1.0 / 32.0)
            x1r = acc_pool.tile([128, PB], F32, tag="x1r")
            nc.vector.scalar_tensor_tensor(
                x1r[:], spi[:], -32.0, mval[:], op0=OP.mult, op1=OP.add
            )
            if tanh_epi:
                # sig chains via tanh (same table set as Exp):
                # sig(u) = (tanh(u/2)+1)/2, and the nested pair folds:
                # sig(10*s-4.5) with s=(t+1)/2  ->  (tanh(2.5*t+0.25)+1)/2
                t1 = acc_pool.tile([128, PB], F32, tag="t1w")
                nc.scalar.activation(t1[:], x1r[:], ACT.Tanh, scale=0.5, bias=bm4[:])
                t1b = acc_pool.tile([128, PB], F32, tag="t1b")
                nc.scalar.activation(t1b[:], t1[:], ACT.Tanh, scale=-2.5, bias=b025[:])
                swl = acc_pool.tile([128, PB], F32, tag="swl")
                nc.vector.tensor_scalar(
                    swl[:], t1b[:], 1.0, 0.5, op0=OP.add, op1=OP.mult
                )
            elif lse:
                x1s = sigmoid_exp("x1s", x1r[:], 1.0, b8[:])       # sig(x1r-8)
                swl = sigmoid_exp("swl", x1s[:], -10.0, bm55e[:])  # sig(5.5-10*x1s)
            else:
                x1s = acc_pool.tile([128, PB], F32, tag="x1s")
                nc.scalar.activation(
                    x1s[:], x1r[:], ACT.Sigmoid, scale=1.0, bias=bm8[:]
                )
                swl = acc_pool.tile([128, PB], F32, tag="swl")
                nc.scalar.activation(
                    swl[:], x1s[:], ACT.Sigmoid, scale=-10.0, bias=b55[:]
                )
            gwl = acc_pool.tile([128, PB], F32, tag="gwl")
            nc.vector.tensor_scalar(
                gwl[:], x1r[:], 8.0 + LOGIT_55, None, op0=OP.is_lt
            )
            rwl = acc_pool.tile([128, PB], F32, tag="rwl")
            nc.vector.scalar_tensor_tensor(
                rwl[:], gwl[:], 1.0, swl[:], op0=OP.add, op1=OP.mult
            )

            # O1 chain from mno
            if tanh_epi:
                t3 = acc_pool.tile([128, PB], F32, tag="t3o")
                nc.scalar.activation(t3[:], mno[:], ACT.Tanh, scale=0.5, bias=bz[:])
                t3b = acc_pool.tile([128, PB], F32, tag="t3b")
                nc.scalar.activation(t3b[:], t3[:], ACT.Tanh, scale=2.5, bias=b025[:])
                s1 = acc_pool.tile([128, PB], F32, tag="s1")
                nc.vector.tensor_scalar(
                    s1[:], t3b[:], 1.0, 0.5, op0=OP.add, op1=OP.mult
                )
            elif lse:
                s1a = sigmoid_exp("s1a", mno[:], 1.0, 0.0)
                s1 = sigmoid_exp("s1", s1a[:], 10.0, b45[:])       # sig(10*s1a-4.5)
            else:
                s1a = acc_pool.tile([128, PB], F32, tag="s1a")
                nc.scalar.activation(s1a[:], mno[:], ACT.Sigmoid)
                s1 = acc_pool.tile([128, PB], F32, tag="s1")
                nc.scalar.activation(
                    s1[:], s1a[:], ACT.Sigmoid, scale=10.0, bias=bm45[:]
                )
            g1 = acc_pool.tile([128, PB], F32, tag="g1")
            nc.vector.tensor_scalar(g1[:], mno[:], -LOGIT_55, None, op0=OP.is_gt)
            r1 = acc_pool.tile([128, PB], F32, tag="r1")
            nc.vector.scalar_tensor_tensor(
                r1[:], g1[:], 1.0, s1[:], op0=OP.add, op1=OP.mult
            )

            # O2 chain from the wrong-col stat (v3: w is the max; v5: w is the
            # exp-sum and lw = ln(w) carries (K*max - C + ln neff))
            if tanh_epi and lse:
                # lg = log2(wsum) via the classic bit-hack (err ~0.04 in log2
                # => ~0.0013 in w); then both sigmoids as folded tanh ops:
                # s2a=(tanh((lg*ln2+C)/(2K))+1)/2, s2=(tanh(2.5*t+0.25)+1)/2
                lg = acc_pool.tile([128, PB], F32, tag="lg")
                nc.vector.tensor_scalar(
                    lg[:], w[:].bitcast(I32), 2.0 ** -23, BITHACK_BIAS,
                    op0=OP.mult, op1=OP.add,
                )
                t2 = acc_pool.tile([128, PB], F32, tag="t2o")
                nc.scalar.activation(
                    t2[:], lg[:], ACT.Tanh,
                    scale=LN2 / (2 * K_LSE), bias=bC2K[:],
                )
                t2b = acc_pool.tile([128, PB], F32, tag="t2b")
                nc.scalar.activation(t2b[:], t2[:], ACT.Tanh, scale=2.5, bias=b025[:])
                s2 = acc_pool.tile([128, PB], F32, tag="s2")
                nc.vector.tensor_scalar(
                    s2[:], t2b[:], 1.0, 0.5, op0=OP.add, op1=OP.mult
                )
                g2 = acc_pool.tile([128, PB], F32, tag="g2")
                nc.vector.tensor_scalar(
                    g2[:], lg[:], (-K_LSE * LOGIT_55 - C_LSE) / LN2, None,
                    op0=OP.is_gt,
                )
            elif tanh_epi:
                t2 = acc_pool.tile([128, PB], F32, tag="t2o")
                nc.scalar.activation(t2[:], w[:], ACT.Tanh, scale=0.5, bias=bz[:])
                t2b = acc_pool.tile([128, PB], F32, tag="t2b")
                nc.scalar.activation(t2b[:], t2[:], ACT.Tanh, scale=2.5, bias=b025[:])
                s2 = acc_pool.tile([128, PB], F32, tag="s2")
                nc.vector.tensor_scalar(
                    s2[:], t2b[:], 1.0, 0.5, op0=OP.add, op1=OP.mult
                )
                g2 = acc_pool.tile([128, PB], F32, tag="g2")
                nc.vector.tensor_scalar(g2[:], w[:], -LOGIT_55, None, op0=OP.is_gt)
            elif lse:
                lw = acc_pool.tile([128, PB], F32, tag="lw")
                nc.scalar.activation(lw[:], w[:], ACT.Ln)
                # s2a = sig((lw+C)/K) = 1/(1+exp(-lw/K - C/K))
                s2a = sigmoid_exp("s2a", lw[:], 1.0 / K_LSE, bmCk[:])
                s2 = sigmoid_exp("s2", s2a[:], 10.0, b45[:])
                g2 = acc_pool.tile([128, PB], F32, tag="g2")
                nc.vector.tensor_scalar(
                    g2[:], lw[:], -K_LSE * LOGIT_55 - C_LSE, None, op0=OP.is_gt
                )
            else:
                s2a = acc_pool.tile([128, PB], F32, tag="s2a")
                nc.scalar.activation(s2a[:], w[:], ACT.Sigmoid)
                s2 = acc_pool.tile([128, PB], F32, tag="s2")
                nc.scalar.activation(
                    s2[:], s2a[:], ACT.Sigmoid, scale=10.0, bias=bm45[:]
                )
                g2 = acc_pool.tile([128, PB], F32, tag="g2")
                nc.vector.tensor_scalar(g2[:], w[:], -LOGIT_55, None, op0=OP.is_gt)
            r2 = acc_pool.tile([128, PB], F32, tag="r2")
            nc.vector.scalar_tensor_tensor(
                r2[:], g2[:], 1.0, s2[:], op0=OP.add, op1=OP.mult
            )

            # merge: loss = has_wl ? rwl : 0.5*(rank1 + rank2)
            ro = acc_pool.tile([128, PB], F32, tag="ro")
            nc.vector.tensor_add(ro[:], r1[:], r2[:])
            nc.vector.tensor_scalar_mul(ro[:], ro[:], 0.5)
            hwl = acc_pool.tile([128, PB], I32, tag="hwl")
            nc.vector.tensor_scalar(hwl[:], mval[:], 16.0, None, op0=OP.is_gt)
            loss = acc_pool.tile([128, PB], F32, tag="loss")
            nc.vector.select(loss[:], hwl[:], rwl[:], ro[:])

            # sum: PE reduces partitions to [1, PB]; ACT copy accumulates to [1,1]
            if True:
                ps = psum_pool.tile([1, PB], F32)
                nc.tensor.matmul(ps[:], ones[:], loss[:], start=True, stop=True)
                res = acc_pool.tile([1, PB], F32, tag="res")
                nc.scalar.copy(res[:], ps[:])
            nc.sync.dma_start(out_ext[:, :], res[:])

    nc.finalize()
    return nc


DEFAULT_VARIANT = "v3"

_NC_CACHE = {}


def _get_nc(variant=None):
    if variant is None:
        variant = DEFAULT_VARIANT
    if variant not in _NC_CACHE:
        _NC_CACHE[variant] = build_nc(variant=variant)
    return _NC_CACHE[variant]


_BF16NP = mybir.dt.np(BF16)


def _col_reorder(a):
    """[R, 1000] -> A|B halves: each group's cols 0:25 then 25:50, contiguous."""
    r = a.reshape(-1, L, 2, G // 2)
    return np.concatenate(
        [np.ascontiguousarray(r[:, :, 0, :]).reshape(-1, L * (G // 2)),
         np.ascontiguousarray(r[:, :, 1, :]).reshape(-1, L * (G // 2))],
        axis=1,
    )


def _dev_layout(a, W):
    """[RPC, W] -> [128, PB*W] block-major partition layout."""
    return np.ascontiguousarray(
        a.reshape(PB, 128, W).transpose(1, 0, 2).reshape(128, PB * W)
    )


_POW25 = (1 << np.arange(G // 2, dtype=np.int64)).astype(np.int64)


def make_in_maps(x, y, y_neg, variant="v3"):
    m_np = _BF16NP if ("w" in variant or variant.startswith("v5") or (variant.startswith("v6") and not variant.startswith("v6x"))) else np.int8
    in_maps = []
    for i in range(NCORES):
        r0 = i * RPC
        xs = _col_reorder(np.asarray(x[r0 : r0 + RPC, :CU])).astype(_BF16NP)
        m = np.where(
            _col_reorder(np.asarray(y_neg[r0 : r0 + RPC, :CU])) != 0, 0, MNEG
        ).astype(m_np)
        bits = (np.asarray(y[r0 : r0 + RPC, :CU]) != 0).reshape(RPC, L, 2, G // 2)
        words = (bits * _POW25).sum(axis=-1).astype(np.int32)  # [RPC, L, 2] < 2^25
        in_maps.append({
            "x": np.ascontiguousarray(xs),
            "m": _dev_layout(m, CU),
            "y_p": _dev_layout(words.reshape(RPC, L * 2), L * 2),
        })
    return in_maps


def kernel(x, y, y_neg, wl_masks=None, **_):
    x = np.asarray(x)
    y = np.asarray(y)
    y_neg = np.asarray(y_neg)
    assert x.shape == (B, C), x.shape
    nc = _get_nc()
    in_maps = make_in_maps(x, y, y_neg, variant=DEFAULT_VARIANT)
    res = run_bass_kernel_spmd(nc, in_maps, core_ids=list(range(NCORES)))
    total = np.float32(0.0)
    for r in res.results:
        total += np.float32(r["out"].reshape(-1).sum())
    return np.float32(total)
